# revision 6
# baseline (speedup 1.0000x reference)
"""Trainium2 Bass kernel for BigVGAN AMPBlock1 (nn_AMPBlock1_81655918231624).

Strategy: data-parallel over batch (B=8 -> 1 sample per NeuronCore).
Per core, the whole block runs channel-major ([128 part = channel mod 128,
4 groups, T]) as 6 sequential "units" (act1d + dilated conv), with DRAM
spill between units:

  - up/down anti-alias FIRs: polyphase 6-tap filters as diagonal-matrix
    f32r matmuls on the TensorEngine (PSUM-accumulated).
  - snakebeta: x + sin^2(a*x)/b via range-reduced Sin on ScalarE:
      q = u * a/(2pi)            (DVE, PSUM read)
      r = (q + M) - M            (DVE, fp32 magic-number round)
      f = q - r  in [-0.5, 0.5]  (DVE)
      s = Sin(2pi * f)           (ScalarE; = +-sin(a*u), sign dies in square)
      p = Square(s * sqrt(1/b))  (ScalarE; = sin^2(a*u)/b)
      z = u + p                  (DVE)
  - 512x512 k=3 dilated convs: f32r matmuls, 4x4 channel blocks x 3 taps.
  - residual adds fused into the conv2 PSUM eviction.

Self-contained: shapes hardcoded; no sibling imports.
"""
import numpy as np

# ---------------------------------------------------------------- constants
B, C, T = 8, 512, 8192
G, P = 4, 128            # channel groups x partitions
KER = 3
DILATIONS = (1, 3, 5)
FILT_K = 12
TC = 256                 # v1 output columns per time-tile
TCV = 512                # v3 tile length
NT = T // TCV
NTILES = T // TC
MAGIC = 12582912.0       # 1.5 * 2**23: fp32 round-to-nearest-int
N_CORES = 8


def _kaiser_sinc_filter1d(cutoff, half_width, kernel_size):
    even = kernel_size % 2 == 0
    half_size = kernel_size // 2
    delta_f = 4 * half_width
    A = 2.285 * (half_size - 1) * np.pi * delta_f + 7.95
    if A > 50.0:
        beta = 0.1102 * (A - 8.7)
    elif A >= 21.0:
        beta = 0.5842 * (A - 21) ** 0.4 + 0.07886 * (A - 21.0)
    else:
        beta = 0.0
    window = np.kaiser(kernel_size, beta)
    if even:
        time = np.arange(-half_size, half_size) + 0.5
    else:
        time = np.arange(kernel_size) - half_size
    if cutoff == 0:
        filt = np.zeros(kernel_size)
    else:
        filt = 2 * cutoff * window * np.sinc(2 * cutoff * time)
        filt = filt / np.sum(filt)
    return filt


def _polyphase_filters():
    """up even: u[2t]   = sum_i fe[i] * xc[t-3+i]
       up odd:  u[2t+1] = sum_i fo[i] * xc[t-2+i]
       down:    y[t] = sum_j fde[j]*ze[t-2+j] + fdo[j]*zo[t-3+j]"""
    up = _kaiser_sinc_filter1d(0.25, 0.3, FILT_K)
    dn = _kaiser_sinc_filter1d(0.25, 0.3, FILT_K)
    wf = 2.0 * up[::-1]
    return wf[0::2], wf[1::2], dn[1::2], dn[0::2]


def _build_nc_v1(has_bias):
    """Builds the Bacc graph. has_bias/has_act: enable general paths."""
    from concourse import bacc, mybir, tile

    f32r = mybir.dt.float32r
    f32 = mybir.dt.float32
    AF = mybir.ActivationFunctionType
    ALU = mybir.AluOpType

    nc = bacc.Bacc("TRN2", target_bir_lowering=False, debug=False,
                   num_devices=N_CORES)

    x_ext = nc.dram_tensor("x", [P, G, T], f32r, kind="ExternalInput").ap()
    # weights: [unit, part=ci%128, k, gi, co(512)]
    w_ext = nc.dram_tensor("w", [6, P, KER, G, C], f32r,
                           kind="ExternalInput").ap()
    # 24 diagonal filter matrices, partition-major: [part, idx, 128]
    dg_ext = nc.dram_tensor("dg", [P, 24, P], f32r, kind="ExternalInput").ap()
    # per-act scalars: [part, unit, g, {a/(2pi), sqrt(1/(b+eps))}]
    sc_ext = nc.dram_tensor("sc", [P, 6, G, 2], f32, kind="ExternalInput").ap()
    bias_ext = nc.dram_tensor("bias", [P, 6, G], f32, kind="ExternalInput").ap()
    out_ext = nc.dram_tensor("out", [P, G, T], f32, kind="ExternalOutput").ap()

    spill = [nc.dram_tensor(f"spill{j}", [P, G, T], f32r, kind="Internal").ap()
             for j in range(5)]
    # unit j: input src, output dst, residual (None if no add)
    unit_src = [x_ext, spill[0], spill[1], spill[2], spill[3], spill[4]]
    unit_dst = [spill[0], spill[1], spill[2], spill[3], spill[4], out_ext]
    unit_res = [None, x_ext, None, spill[1], None, spill[3]]

    with tile.TileContext(nc) as tc:
        with tc.tile_pool(name="const", bufs=1) as cpool, \
             tc.tile_pool(name="wpool", bufs=2) as wpool, \
             tc.tile_pool(name="sbuf", bufs=3) as pool, \
             tc.tile_pool(name="psum", bufs=1, space="PSUM") as psp:

            dg = cpool.tile([P, 24, P], f32r)
            nc.sync.dma_start(dg[:], dg_ext[:])
            sc = cpool.tile([P, 6, G, 2], f32)
            nc.sync.dma_start(sc[:], sc_ext[:])
            bias_t = cpool.tile([P, 6, G], f32)
            if has_bias:
                nc.sync.dma_start(bias_t[:], bias_ext[:])

            for j in range(6):
                d = DILATIONS[j // 2] if j % 2 == 0 else 1
                h = d + 6                 # left halo in x (col0 = t0 - h)
                XL = TC + 2 * d + 12      # x tile length
                L = TC + 2 * d + 6        # phase (u/z) length, even
                SA = TC + 2 * d           # act output length
                src, dst, res = unit_src[j], unit_dst[j], unit_res[j]

                wt = wpool.tile([P, KER, G, C], f32r, name=f"wt{j}", tag="wt")
                nc.sync.dma_start(wt[:], w_ext[j])

                for i in range(NTILES):
                    t0 = i * TC
                    lo = t0 - h              # absolute x index of x_in col 0
                    x_in = pool.tile([P, G, XL], f32r, name=f"xin{j}_{i}",
                                     tag="xin", bufs=2)
                    # ---- input DMA with edge clamping
                    lo_c = max(lo, 0)
                    hi_c = min(lo + XL, T)
                    nc.sync.dma_start(x_in[:, :, lo_c - lo:hi_c - lo],
                                      src[:, :, lo_c:hi_c])
                    for c in range(lo_c - lo):                    # left clamp
                        nc.sync.dma_start(x_in[:, :, c:c + 1], src[:, :, 0:1])
                    for c in range(hi_c - lo, XL):               # right clamp
                        nc.sync.dma_start(x_in[:, :, c:c + 1],
                                          src[:, :, T - 1:T])

                    if res is not None:
                        res_t = pool.tile([P, G, TC], f32r,
                                          name=f"res{j}_{i}", tag="res")
                        nc.sync.dma_start(res_t[:], res[:, :, t0:t0 + TC])

                    # ---- act1d: up (diag matmuls) + snake + down
                    z_ph = []
                    for ph, base in ((0, 0), (1, 6)):
                        z_t = pool.tile([P, G, L], f32r,
                                        name=f"z{j}_{i}_{ph}", tag=f"z{ph}")
                        for g in range(G):
                            pu = psp.tile([P, L], f32, name=f"pu{j}_{i}_{ph}_{g}",
                                          tag="pu", bufs=4)
                            for ii in range(6):
                                nc.tensor.matmul(pu[:], dg[:, base + ii, :],
                                                 x_in[:, g, ii + 1:ii + 1 + L],
                                                 start=(ii == 0), stop=(ii == 5))
                            q_t = pool.tile([P, L], f32, name=f"q{j}_{i}_{ph}_{g}",
                                            tag="q", bufs=2)
                            nc.vector.tensor_scalar_mul(q_t[:], pu[:],
                                                        sc[:, j, g, 0:1])
                            r_t = pool.tile([P, L], f32, name=f"r{j}_{i}_{ph}_{g}",
                                            tag="r", bufs=2)
                            nc.vector.tensor_scalar(r_t[:], q_t[:], MAGIC, MAGIC,
                                                    op0=ALU.add, op1=ALU.subtract)
                            f_t = pool.tile([P, L], f32, name=f"f{j}_{i}_{ph}_{g}",
                                            tag="f", bufs=2)
                            nc.vector.tensor_sub(f_t[:], q_t[:], r_t[:])
                            s_t = pool.tile([P, L], f32, name=f"s{j}_{i}_{ph}_{g}",
                                            tag="s", bufs=2)
                            nc.scalar.activation(s_t[:], f_t[:], AF.Sin,
                                                 bias=0.0, scale=float(2 * np.pi))
                            p_t = pool.tile([P, L], f32, name=f"p{j}_{i}_{ph}_{g}",
                                            tag="p", bufs=2)
                            nc.scalar.activation(p_t[:], s_t[:], AF.Square,
                                                 bias=0.0, scale=sc[:, j, g, 1:2])
                            nc.vector.tensor_add(z_t[:, g, :], pu[:], p_t[:])
                        z_ph.append(z_t)
                    z_e, z_o = z_ph

                    # ---- z edge clamping (replicate-pad semantics of down)
                    # z_e col c is z-phase-e index mE + c, mE = t0 - d - 2
                    # z_o col c is z-phase-o index mO + c, mO = t0 - d - 3
                    mE = t0 - d - 2
                    mO = t0 - d - 3
                    if i == 0:
                        srcc = -mE        # col of z_e[m=0]
                        for c in range(-mE):          # z_e[m<0] = z_e[0]
                            nc.vector.tensor_copy(z_e[:, :, c:c + 1],
                                                  z_e[:, :, srcc:srcc + 1])
                        for c in range(-mO):          # z_o[m<0] = z_e[0]
                            nc.vector.tensor_copy(z_o[:, :, c:c + 1],
                                                  z_e[:, :, srcc:srcc + 1])
                    if i == NTILES - 1:
                        srco = T - 1 - mO  # col of z_o[m=T-1]
                        for c in range(T - mE, L):    # z_e[m>=T] = z_o[T-1]
                            nc.vector.tensor_copy(z_e[:, :, c:c + 1],
                                                  z_o[:, :, srco:srco + 1])
                        for c in range(T - mO, L):    # z_o[m>=T] = z_o[T-1]
                            nc.vector.tensor_copy(z_o[:, :, c:c + 1],
                                                  z_o[:, :, srco:srco + 1])

                    y_act = pool.tile([P, G, SA], f32r, name=f"ya{j}_{i}",
                                      tag="ya")
                    for g in range(G):
                        pd = psp.tile([P, SA], f32, name=f"pd{j}_{i}_{g}",
                                      tag="pd", bufs=2)
                        for jj in range(6):
                            nc.tensor.matmul(pd[:], dg[:, 12 + jj, :],
                                             z_e[:, g, jj:jj + SA],
                                             start=(jj == 0), stop=False)
                        for jj in range(6):
                            nc.tensor.matmul(pd[:], dg[:, 18 + jj, :],
                                             z_o[:, g, jj:jj + SA],
                                             start=False, stop=(jj == 5))
                        nc.scalar.activation(y_act[:, g, :], pd[:], AF.Copy)

                    # conv zero-padding: act output t<0 or t>=T must be 0
                    if i == 0 and d > 0:
                        nc.vector.memset(y_act[:, :, 0:d].bitcast(f32), 0.0)
                    if i == NTILES - 1 and d > 0:
                        nc.vector.memset(y_act[:, :, SA - d:SA].bitcast(f32), 0.0)

                    # ---- dilated conv 512x512 k=3
                    out_t = pool.tile([P, G, TC], f32r if j < 5 else f32,
                                      name=f"ot{j}_{i}", tag="ot")
                    for go in range(G):
                        pc = psp.tile([P, TC], f32, name=f"pc{j}_{i}_{go}",
                                      tag="pc", bufs=2)
                        first = True
                        for k in range(KER):
                            for gi in range(G):
                                nc.tensor.matmul(
                                    pc[:], wt[:, k, gi, go * P:(go + 1) * P],
                                    y_act[:, gi, k * d:k * d + TC],
                                    start=first, stop=(k == KER - 1 and gi == G - 1))
                                first = False
                        if res is not None:
                            if has_bias:
                                tmp = pool.tile([P, TC], f32, name=f"tb{j}_{i}_{go}",
                                                tag="tb", bufs=2)
                                nc.scalar.activation(tmp[:], pc[:], AF.Identity,
                                                     bias=bias_t[:, j, go:go + 1])
                                nc.vector.tensor_add(
                                    out_t[:, go, :], tmp[:],
                                    res_t[:, go, :].bitcast(f32))
                            else:
                                nc.vector.tensor_add(
                                    out_t[:, go, :], pc[:],
                                    res_t[:, go, :].bitcast(f32))
                        else:
                            if has_bias:
                                nc.scalar.activation(out_t[:, go, :], pc[:],
                                                     AF.Identity,
                                                     bias=bias_t[:, j, go:go + 1])
                            else:
                                nc.scalar.activation(out_t[:, go, :], pc[:],
                                                     AF.Copy)
                    nc.sync.dma_start(dst[:, :, t0:t0 + TC], out_t[:])
    nc.compile()
    return nc


def v3_host_mats():
    """TUP [128,128], TDN [128,3,128], IDN [128,128], all fp16."""
    fe, fo = _polyphase_filters()[:2]
    df = _kaiser_sinc_filter1d(0.25, 0.3, FILT_K)
    tup = np.zeros((P, P), dtype=np.float64)
    for r in range(P):
        if r % 2 == 0:
            for i in range(6):
                tup[r // 2 + i, r] = fe[i]
        else:
            for i in range(6):
                tup[(r - 1) // 2 + 1 + i, r] = fo[i]
    tdn = np.zeros((P, 3, P), dtype=np.float64)
    for k in range(3):
        for zr in range(P):
            for yr in range(P):
                jj = 128 * k + zr - 2 * yr - 1
                if 0 <= jj < FILT_K:
                    tdn[zr, k, yr] = df[jj]
    idn = np.eye(P)
    return (tup.astype(np.float16), tdn.astype(np.float16),
            idn.astype(np.float16))


def build_nc_v3(act_consts):
    """act_consts: [(a2pi_j, sb_j)] * 6, python floats."""
    from concourse import bacc, mybir, tile

    f16 = mybir.dt.float16
    f32 = mybir.dt.float32
    AF = mybir.ActivationFunctionType
    ALU = mybir.AluOpType

    nc = bacc.Bacc("TRN2", target_bir_lowering=False, debug=False,
                   num_devices=N_CORES)

    NB = T // P  # 64 row-blocks, block-major DRAM: [part, blk, C]
    x_ext = nc.dram_tensor("x", [P, NB, C], f16, kind="ExternalInput").ap()
    w_ext = nc.dram_tensor("w", [6, P, KER, G, C], f16,
                           kind="ExternalInput").ap()
    tup_ext = nc.dram_tensor("tup", [P, P], f16, kind="ExternalInput").ap()
    tdn_ext = nc.dram_tensor("tdn", [P, 3, P], f16,
                             kind="ExternalInput").ap()
    idn_ext = nc.dram_tensor("idn", [P, P], f16, kind="ExternalInput").ap()
    out_ext = nc.dram_tensor("out", [P, NB, C], f32,
                             kind="ExternalOutput").ap()
    spill = [nc.dram_tensor(f"spill{j}", [P, NB, C], f16,
                            kind="Internal").ap()
             for j in range(5)]
    unit_src = [x_ext, spill[0], spill[1], spill[2], spill[3], spill[4]]
    unit_dst = [spill[0], spill[1], spill[2], spill[3], spill[4], out_ext]
    unit_res = [None, x_ext, None, spill[1], None, spill[3]]

    with tile.TileContext(nc) as tc:
        with tc.tile_pool(name="const", bufs=1) as cpool, \
             tc.tile_pool(name="wpool", bufs=2) as wpool, \
             tc.tile_pool(name="sbuf", bufs=3) as pool, \
             tc.tile_pool(name="psum", bufs=1, space="PSUM") as psp:

            tup = cpool.tile([P, P], f16)
            nc.sync.dma_start(tup[:], tup_ext[:])
            tdn = cpool.tile([P, 3, P], f16)
            nc.sync.dma_start(tdn[:], tdn_ext[:])
            idn = cpool.tile([P, P], f16)
            nc.sync.dma_start(idn[:], idn_ext[:])

            pend = {"v": None}  # pending conv (tile i-1, may cross units)
            def dma_sp(dst, srcap, chunks=4):
                """Bulk HBM load via gpsimd SWDGE: sprays descriptors across
                all 16 SDMA engines (the sync HWDGE ring serializes big
                HBM->SBUF loads onto one engine at ~25 GB/s)."""
                nc.gpsimd.dma_start(dst, srcap)

            def emit_conv(part):
                pv = pend["v"]
                if pv is None:
                    return
                jj, ii = pv["j"], pv["i"]
                ycm_p, wt_p, rt_p = pv["ycm"], pv["wt"], pv["res"]
                dst_p, dd = pv["dst"], pv["d"]
                t0p = ii * TCV
                if part == "early":
                    pv["ot"] = pool.tile([P, G, C], f16 if jj < 5 else f32,
                                         name=f"ot{jj}_{ii}",
                                         tag="ot" if jj < 5 else "ot32",
                                         bufs=2)
                    pv["pc"] = []
                    for tb in range(G):
                        pc = psp.tile([P, C], f32, name=f"pc{jj}_{ii}_{tb}",
                                      tag="pc", bufs=2)
                        pv["pc"].append(pc)
                        first = True
                        if rt_p is not None:
                            nc.tensor.matmul(pc[:], idn[:], rt_p[:, tb, :],
                                             start=True, stop=False)
                            first = False
                        for k in range(KER):
                            if tb == G - 1 and k == KER - 1:
                                break  # needs right halo; emitted in 'late'
                            for gi in range(G):
                                stop = (tb < G - 1 and k == KER - 1
                                        and gi == G - 1)
                                nc.tensor.matmul(
                                    pc[:],
                                    ycm_p[:, gi, 128 * tb + k * dd:
                                          128 * tb + k * dd + 128],
                                    wt_p[:, k, gi, :],
                                    start=first, stop=stop)
                                first = False
                        if tb < G - 1:
                            nc.scalar.activation(pv["ot"][:, tb, :], pc[:],
                                                 AF.Copy)
                else:  # 'late'
                    pc = pv["pc"][G - 1]
                    tb = G - 1
                    for gi in range(G):
                        nc.tensor.matmul(
                            pc[:],
                            ycm_p[:, gi, 128 * tb + 2 * dd:
                                  128 * tb + 2 * dd + 128],
                            wt_p[:, 2, gi, :],
                            start=False, stop=(gi == G - 1))
                    nc.scalar.activation(pv["ot"][:, tb, :], pc[:], AF.Copy)
                    bk = 4 * ii
                    nc.scalar.dma_start(dst_p[0:64, bk:bk + 4, :],
                                        pv["ot"][0:64])
                    nc.scalar.dma_start(dst_p[64:128, bk:bk + 4, :],
                                        pv["ot"][64:128])
                    pend["v"] = None

            for j in range(6):
                d = DILATIONS[j // 2] if j % 2 == 0 else 1
                a2pi, sb = act_consts[j]
                src, dst, res = unit_src[j], unit_dst[j], unit_res[j]

                wt = wpool.tile([P, KER, G, C], f16, name=f"wt{j}", tag="wt")
                nc.gpsimd.dma_start(wt[:], w_ext[j])

                prev_xe = None
                prev_z = None
                prev_y = None

                for i in range(NT):
                    t0 = i * TCV
                    # ---------------- loads ----------------
                    bk = 4 * i
                    xe = pool.tile([P, 5, C], f16, name=f"xe{j}_{i}",
                                   tag="xe", bufs=2)
                    if i == 0:
                        dma_sp(xe[6:128, 0:5, :], src[0:122, 0:5, :])
                        nc.gpsimd.dma_start(xe[0:6, 1:5, :],
                                            src[122:128, 0:4, :])
                        for p_ in range(6):
                            nc.sync.dma_start(xe[p_:p_ + 1, 0, :],
                                              src[0:1, 0, :])
                    elif i < NT - 1:
                        dma_sp(xe[6:128, 0:5, :], src[0:122, bk:bk + 5, :])
                        nc.gpsimd.dma_start(xe[0:6, 0:5, :],
                                            src[122:128, bk - 1:bk + 4, :])
                    else:
                        nc.vector.memset(xe[:, 4, :], 0.0)
                        dma_sp(xe[6:128, 0:4, :], src[0:122, bk:bk + 4, :])
                        nc.gpsimd.dma_start(xe[0:6, 0:5, :],
                                            src[122:128, bk - 1:bk + 4, :])
                        for p_ in range(6, 9):
                            nc.sync.dma_start(xe[p_:p_ + 1, 4, :],
                                              src[127:128, NB - 1, :])
                    xo = pool.tile([P, 4, C], f16, name=f"xo{j}_{i}",
                                   tag="xo", bufs=2)
                    if i < NT - 1:
                        dma_sp(xo[0:70, 0:4, :], src[58:128, bk:bk + 4, :],
                               chunks=3)
                        dma_sp(xo[70:128, 0:4, :],
                               src[0:58, bk + 1:bk + 5, :], chunks=2)
                    else:
                        nc.vector.memset(xo[:, 3, :], 0.0)
                        dma_sp(xo[0:70, 0:4, :], src[58:128, bk:bk + 4, :],
                               chunks=3)
                        dma_sp(xo[70:128, 0:3, :],
                               src[0:58, bk + 1:bk + 4, :], chunks=2)
                    rt = None
                    if res is not None:
                        rt = pool.tile([P, G, C], f16, name=f"rt{j}_{i}",
                                       tag="rt", bufs=2)
                        dma_sp(rt[:], res[:, bk:bk + 4, :], chunks=3)

                    ycm = pool.tile([P, G, TCV + 10], f16,
                                    name=f"ycm{j}_{i}", tag="ycm", bufs=3)
                    if i == 0:
                        nc.vector.memset(ycm[:, :, 0:d], 0.0)
                    else:
                        nc.sync.dma_start(ycm[:, :, 0:d],
                                          prev_y[:, :, TCV:TCV + d])

                    # ------------- up + snake (+ conv early) -------------
                    z_tm = pool.tile([P, 9, C], f16, name=f"z{j}_{i}",
                                     tag="z", bufs=2)
                    ms = list(range(0, 9)) if i == 0 else list(range(1, 9))
                    zq = []  # (m, pu, p_t) awaiting the z-add

                    def flush_z():
                        while zq:
                            m0, pu0, p0 = zq.pop(0)
                            nc.vector.tensor_add(z_tm[:, m0, :], pu0[:],
                                                 p0[:])

                    def snake(m):
                        # z = u + sin^2(a u)/b = u + c0 - c0*cos(2 a u),
                        # c0 = 1/(2b).  cos(2au) = sin(2pi*frac(a u/pi + .25))
                        pu = psp.tile([P, C], f32, name=f"pu{j}_{i}_{m}",
                                      tag="pu", bufs=4)
                        mov = xe[:, m // 2, :] if m % 2 == 0 \
                            else xo[:, (m - 1) // 2, :]
                        nc.tensor.matmul(pu[:], tup[:], mov,
                                         start=True, stop=True)
                        q_t = pool.tile([P, C], f32, name=f"q{j}_{i}_{m}",
                                        tag="q", bufs=3)
                        nc.vector.tensor_scalar(q_t[:], pu[:], a2pi, 0.25,
                                                op0=ALU.mult, op1=ALU.add)
                        r_t = pool.tile([P, C], f32, name=f"r{j}_{i}_{m}",
                                        tag="r", bufs=3)
                        nc.vector.tensor_scalar(r_t[:], q_t[:], MAGIC, MAGIC,
                                                op0=ALU.add, op1=ALU.subtract)
                        f_t = pool.tile([P, C], f32, name=f"f{j}_{i}_{m}",
                                        tag="f", bufs=3)
                        nc.gpsimd.tensor_sub(f_t[:], q_t[:], r_t[:])
                        s_t = pool.tile([P, C], f32, name=f"s{j}_{i}_{m}",
                                        tag="s", bufs=3)
                        nc.scalar.activation(s_t[:], f_t[:], AF.Sin,
                                             bias=0.0,
                                             scale=float(2 * np.pi))
                        p_t = pool.tile([P, C], f32, name=f"p{j}_{i}_{m}",
                                        tag="p", bufs=3)
                        nc.vector.tensor_scalar(p_t[:], s_t[:], -sb, sb,
                                                op0=ALU.mult, op1=ALU.add)
                        if zq:
                            m0, pu0, p0 = zq.pop(0)
                            nc.vector.tensor_add(z_tm[:, m0, :], pu0[:],
                                                 p0[:])
                        zq.append((m, pu, p_t))

                    nsplit = 5 if i == 0 else 4
                    for m in ms[:nsplit]:
                        snake(m)
                    emit_conv("early")
                    for m in ms[nsplit:]:
                        snake(m)
                    flush_z()

                    # edge clamps on z
                    if i == 0:
                        for p_ in range(6):
                            nc.sync.dma_start(z_tm[p_:p_ + 1, 0, :],
                                              z_tm[6:7, 0, :])
                    if i == NT - 1:
                        bz = pool.tile([P, C], f16, name=f"bz{j}",
                                       tag="bz", bufs=1)
                        nc.sync.dma_start(bz[0:1, :], z_tm[5:6, 8, :])
                        nc.gpsimd.partition_broadcast(bz[:], bz[0:1, :])
                        nc.sync.dma_start(z_tm[6:128, 8, :], bz[6:128, :])

                    # ---------------- down ----------------
                    for b in range(G):
                        pd = psp.tile([P, C], f32, name=f"pd{j}_{i}_{b}",
                                      tag="pd", bufs=2)
                        for n in range(G):
                            for k in range(3):
                                zblk = 2 * n + k
                                if zblk == 0 and i > 0:
                                    zsrc = prev_z[:, 8,
                                                  128 * b:128 * (b + 1)]
                                else:
                                    zsrc = z_tm[:, zblk,
                                                128 * b:128 * (b + 1)]
                                nc.tensor.matmul(
                                    pd[:, 128 * n:128 * (n + 1)],
                                    zsrc, tdn[:, k, :],
                                    start=(k == 0), stop=(k == 2))
                        nc.scalar.activation(ycm[:, b, d:TCV + d], pd[:],
                                             AF.Copy)

                    if i == NT - 1:
                        nc.vector.memset(ycm[:, :, TCV + d:TCV + 2 * d], 0.0)
                    if pend["v"] is not None:
                        pv = pend["v"]
                        if pv["j"] == j:  # same unit: fill right halo
                            nc.sync.dma_start(
                                pv["ycm"][:, :,
                                          TCV + pv["d"]:TCV + 2 * pv["d"]],
                                ycm[:, :, d:2 * d])
                        emit_conv("late")

                    pend["v"] = {"j": j, "i": i, "ycm": ycm, "wt": wt,
                                 "res": rt, "dst": dst, "d": d}
                    prev_xe, prev_z, prev_y = xe, z_tm, ycm

            emit_conv("early")
            emit_conv("late")
    nc.compile()
    return nc

_NC_CACHE = {}
LAST_EXEC_NS = None


def kernel(**inputs):
    from concourse.bass_utils import run_bass_kernel_spmd

    x = np.asarray(inputs["x"], dtype=np.float32)          # [B, C, T]
    fe, fo, fde, fdo = _polyphase_filters()

    # diag filter matrices [P, 24, P] (v1 fallback)
    dg = np.zeros((P, 24, P), dtype=np.float32)
    coeffs = list(fe) + list(fo) + list(fde) + list(fdo)
    for idx, cf in enumerate(coeffs):
        np.fill_diagonal(dg[:, idx, :], cf)

    # weights [6, P, KER, G, C]: unit 2l -> w1_l, unit 2l+1 -> w2_l
    w_all = np.zeros((6, P, KER, G, C), dtype=np.float32)
    b_all = np.zeros((P, 6, G), dtype=np.float32)
    for l in range(3):
        for half, nm in ((0, "w1"), (1, "w2")):
            j = 2 * l + half
            w = np.asarray(inputs[f"{nm}_{l}"], dtype=np.float32)
            w_all[j] = w.transpose(1, 2, 0).reshape(G, P, KER, C).transpose(
                1, 2, 0, 3)
            b = np.asarray(inputs[f"b{half + 1}_{l}"], dtype=np.float32)
            b_all[:, j, :] = b.reshape(G, P).T

    sc = np.zeros((P, 6, G, 2), dtype=np.float32)
    for j in range(6):
        a = np.exp(np.asarray(inputs[f"alpha_{j}"], dtype=np.float64))
        bb = np.exp(np.asarray(inputs[f"beta_{j}"], dtype=np.float64))
        sc[:, j, :, 0] = (a / (2 * np.pi)).reshape(G, P).T
        sc[:, j, :, 1] = (1.0 / np.sqrt(bb + 1e-9)).reshape(G, P).T

    has_bias = bool(np.any(b_all != 0.0))
    uni = (not has_bias) and all(
        np.ptp(np.asarray(inputs[f"alpha_{j}"])) == 0
        and np.ptp(np.asarray(inputs[f"beta_{j}"])) == 0
        for j in range(6))
    global LAST_EXEC_NS
    if uni:
        act_consts = []
        for j in range(6):
            a = float(np.exp(np.asarray(inputs[f"alpha_{j}"]).ravel()[0]))
            bb = float(np.exp(np.asarray(inputs[f"beta_{j}"]).ravel()[0]))
            act_consts.append((a / np.pi, 1.0 / (2.0 * (bb + 1e-9))))
        key = ("v3", tuple(act_consts))
        if key not in _NC_CACHE:
            _NC_CACHE[key] = build_nc_v3(act_consts)
        nc = _NC_CACHE[key]
        tupm, tdnm, idnm = v3_host_mats()
        w16 = w_all.astype(np.float16)
        in_maps = []
        for bi in range(B):
            xb = x[bi].T.astype(np.float16)                       # [T, C]
            xb = np.ascontiguousarray(
                xb.reshape(T // P, P, C).transpose(1, 0, 2))      # [P,NB,C]
            in_maps.append({"x": xb, "w": w16, "tup": tupm, "tdn": tdnm,
                            "idn": idnm})
        res = run_bass_kernel_spmd(nc, in_maps, core_ids=list(range(N_CORES)))
        LAST_EXEC_NS = res.exec_time_ns
        out = np.empty((B, C, T), dtype=np.float32)
        for bi in range(B):
            ob = res.results[bi]["out"]                           # [P,NB,C]
            out[bi] = ob.transpose(1, 0, 2).reshape(T, C).T
        return out

    key = ("v1", has_bias)
    if key not in _NC_CACHE:
        _NC_CACHE[key] = _build_nc_v1(has_bias)
    nc = _NC_CACHE[key]
    in_maps = []
    for bi in range(B):
        xb = x[bi].reshape(G, P, T).transpose(1, 0, 2).copy()
        in_maps.append({"x": xb, "w": w_all, "dg": dg, "sc": sc,
                        "bias": b_all})
    res = run_bass_kernel_spmd(nc, in_maps, core_ids=list(range(N_CORES)))
    LAST_EXEC_NS = res.exec_time_ns
    out = np.empty((B, C, T), dtype=np.float32)
    for bi in range(B):
        out[bi] = res.results[bi]["out"].transpose(1, 0, 2).reshape(C, T)
    return out



# revision 16
# speedup vs baseline: 1.0685x; 1.0685x over previous
"""Trainium2 Bass kernel for BigVGAN AMPBlock1 (nn_AMPBlock1_81655918231624).

Strategy: data-parallel over batch (B=8 -> 1 sample per NeuronCore).
Per core, the whole block runs channel-major ([128 part = channel mod 128,
4 groups, T]) as 6 sequential "units" (act1d + dilated conv), with DRAM
spill between units:

  - up/down anti-alias FIRs: polyphase 6-tap filters as diagonal-matrix
    f32r matmuls on the TensorEngine (PSUM-accumulated).
  - snakebeta: x + sin^2(a*x)/b via range-reduced Sin on ScalarE:
      q = u * a/(2pi)            (DVE, PSUM read)
      r = (q + M) - M            (DVE, fp32 magic-number round)
      f = q - r  in [-0.5, 0.5]  (DVE)
      s = Sin(2pi * f)           (ScalarE; = +-sin(a*u), sign dies in square)
      p = Square(s * sqrt(1/b))  (ScalarE; = sin^2(a*u)/b)
      z = u + p                  (DVE)
  - 512x512 k=3 dilated convs: f32r matmuls, 4x4 channel blocks x 3 taps.
  - residual adds fused into the conv2 PSUM eviction.

Self-contained: shapes hardcoded; no sibling imports.
"""
import numpy as np

# ---------------------------------------------------------------- constants
B, C, T = 8, 512, 8192
G, P = 4, 128            # channel groups x partitions
KER = 3
DILATIONS = (1, 3, 5)
FILT_K = 12
TC = 256                 # v1 output columns per time-tile
TCV = 512                # v3 tile length
NT = T // TCV
NTILES = T // TC
MAGIC = 12582912.0       # 1.5 * 2**23: fp32 round-to-nearest-int
MAGIC16 = 1536.0         # 1.5 * 2**10: f16 round-to-nearest-int
N_CORES = 8


def _kaiser_sinc_filter1d(cutoff, half_width, kernel_size):
    even = kernel_size % 2 == 0
    half_size = kernel_size // 2
    delta_f = 4 * half_width
    A = 2.285 * (half_size - 1) * np.pi * delta_f + 7.95
    if A > 50.0:
        beta = 0.1102 * (A - 8.7)
    elif A >= 21.0:
        beta = 0.5842 * (A - 21) ** 0.4 + 0.07886 * (A - 21.0)
    else:
        beta = 0.0
    window = np.kaiser(kernel_size, beta)
    if even:
        time = np.arange(-half_size, half_size) + 0.5
    else:
        time = np.arange(kernel_size) - half_size
    if cutoff == 0:
        filt = np.zeros(kernel_size)
    else:
        filt = 2 * cutoff * window * np.sinc(2 * cutoff * time)
        filt = filt / np.sum(filt)
    return filt


def _polyphase_filters():
    """up even: u[2t]   = sum_i fe[i] * xc[t-3+i]
       up odd:  u[2t+1] = sum_i fo[i] * xc[t-2+i]
       down:    y[t] = sum_j fde[j]*ze[t-2+j] + fdo[j]*zo[t-3+j]"""
    up = _kaiser_sinc_filter1d(0.25, 0.3, FILT_K)
    dn = _kaiser_sinc_filter1d(0.25, 0.3, FILT_K)
    wf = 2.0 * up[::-1]
    return wf[0::2], wf[1::2], dn[1::2], dn[0::2]


def _build_nc_v1(has_bias):
    """Builds the Bacc graph. has_bias/has_act: enable general paths."""
    from concourse import bacc, mybir, tile

    f32r = mybir.dt.float32r
    f32 = mybir.dt.float32
    AF = mybir.ActivationFunctionType
    ALU = mybir.AluOpType

    nc = bacc.Bacc("TRN2", target_bir_lowering=False, debug=False,
                   num_devices=N_CORES)

    x_ext = nc.dram_tensor("x", [P, G, T], f32r, kind="ExternalInput").ap()
    # weights: [unit, part=ci%128, k, gi, co(512)]
    w_ext = nc.dram_tensor("w", [6, P, KER, G, C], f32r,
                           kind="ExternalInput").ap()
    # 24 diagonal filter matrices, partition-major: [part, idx, 128]
    dg_ext = nc.dram_tensor("dg", [P, 24, P], f32r, kind="ExternalInput").ap()
    # per-act scalars: [part, unit, g, {a/(2pi), sqrt(1/(b+eps))}]
    sc_ext = nc.dram_tensor("sc", [P, 6, G, 2], f32, kind="ExternalInput").ap()
    bias_ext = nc.dram_tensor("bias", [P, 6, G], f32, kind="ExternalInput").ap()
    out_ext = nc.dram_tensor("out", [P, G, T], f32, kind="ExternalOutput").ap()

    spill = [nc.dram_tensor(f"spill{j}", [P, G, T], f32r, kind="Internal").ap()
             for j in range(5)]
    # unit j: input src, output dst, residual (None if no add)
    unit_src = [x_ext, spill[0], spill[1], spill[2], spill[3], spill[4]]
    unit_dst = [spill[0], spill[1], spill[2], spill[3], spill[4], out_ext]
    unit_res = [None, x_ext, None, spill[1], None, spill[3]]

    with tile.TileContext(nc) as tc:
        with tc.tile_pool(name="const", bufs=1) as cpool, \
             tc.tile_pool(name="wpool", bufs=2) as wpool, \
             tc.tile_pool(name="sbuf", bufs=3) as pool, \
             tc.tile_pool(name="psum", bufs=1, space="PSUM") as psp:

            dg = cpool.tile([P, 24, P], f32r)
            nc.sync.dma_start(dg[:], dg_ext[:])
            sc = cpool.tile([P, 6, G, 2], f32)
            nc.sync.dma_start(sc[:], sc_ext[:])
            bias_t = cpool.tile([P, 6, G], f32)
            if has_bias:
                nc.sync.dma_start(bias_t[:], bias_ext[:])

            for j in range(6):
                d = DILATIONS[j // 2] if j % 2 == 0 else 1
                h = d + 6                 # left halo in x (col0 = t0 - h)
                XL = TC + 2 * d + 12      # x tile length
                L = TC + 2 * d + 6        # phase (u/z) length, even
                SA = TC + 2 * d           # act output length
                src, dst, res = unit_src[j], unit_dst[j], unit_res[j]

                wt = wpool.tile([P, KER, G, C], f32r, name=f"wt{j}", tag="wt")
                nc.sync.dma_start(wt[:], w_ext[j])

                for i in range(NTILES):
                    t0 = i * TC
                    lo = t0 - h              # absolute x index of x_in col 0
                    x_in = pool.tile([P, G, XL], f32r, name=f"xin{j}_{i}",
                                     tag="xin", bufs=2)
                    # ---- input DMA with edge clamping
                    lo_c = max(lo, 0)
                    hi_c = min(lo + XL, T)
                    nc.sync.dma_start(x_in[:, :, lo_c - lo:hi_c - lo],
                                      src[:, :, lo_c:hi_c])
                    for c in range(lo_c - lo):                    # left clamp
                        nc.sync.dma_start(x_in[:, :, c:c + 1], src[:, :, 0:1])
                    for c in range(hi_c - lo, XL):               # right clamp
                        nc.sync.dma_start(x_in[:, :, c:c + 1],
                                          src[:, :, T - 1:T])

                    if res is not None:
                        res_t = pool.tile([P, G, TC], f32r,
                                          name=f"res{j}_{i}", tag="res")
                        nc.sync.dma_start(res_t[:], res[:, :, t0:t0 + TC])

                    # ---- act1d: up (diag matmuls) + snake + down
                    z_ph = []
                    for ph, base in ((0, 0), (1, 6)):
                        z_t = pool.tile([P, G, L], f32r,
                                        name=f"z{j}_{i}_{ph}", tag=f"z{ph}")
                        for g in range(G):
                            pu = psp.tile([P, L], f32, name=f"pu{j}_{i}_{ph}_{g}",
                                          tag="pu", bufs=4)
                            for ii in range(6):
                                nc.tensor.matmul(pu[:], dg[:, base + ii, :],
                                                 x_in[:, g, ii + 1:ii + 1 + L],
                                                 start=(ii == 0), stop=(ii == 5))
                            q_t = pool.tile([P, L], f32, name=f"q{j}_{i}_{ph}_{g}",
                                            tag="q", bufs=2)
                            nc.vector.tensor_scalar_mul(q_t[:], pu[:],
                                                        sc[:, j, g, 0:1])
                            r_t = pool.tile([P, L], f32, name=f"r{j}_{i}_{ph}_{g}",
                                            tag="r", bufs=2)
                            nc.vector.tensor_scalar(r_t[:], q_t[:], MAGIC, MAGIC,
                                                    op0=ALU.add, op1=ALU.subtract)
                            f_t = pool.tile([P, L], f32, name=f"f{j}_{i}_{ph}_{g}",
                                            tag="f", bufs=2)
                            nc.vector.tensor_sub(f_t[:], q_t[:], r_t[:])
                            s_t = pool.tile([P, L], f32, name=f"s{j}_{i}_{ph}_{g}",
                                            tag="s", bufs=2)
                            nc.scalar.activation(s_t[:], f_t[:], AF.Sin,
                                                 bias=0.0, scale=float(2 * np.pi))
                            p_t = pool.tile([P, L], f32, name=f"p{j}_{i}_{ph}_{g}",
                                            tag="p", bufs=2)
                            nc.scalar.activation(p_t[:], s_t[:], AF.Square,
                                                 bias=0.0, scale=sc[:, j, g, 1:2])
                            nc.vector.tensor_add(z_t[:, g, :], pu[:], p_t[:])
                        z_ph.append(z_t)
                    z_e, z_o = z_ph

                    # ---- z edge clamping (replicate-pad semantics of down)
                    # z_e col c is z-phase-e index mE + c, mE = t0 - d - 2
                    # z_o col c is z-phase-o index mO + c, mO = t0 - d - 3
                    mE = t0 - d - 2
                    mO = t0 - d - 3
                    if i == 0:
                        srcc = -mE        # col of z_e[m=0]
                        for c in range(-mE):          # z_e[m<0] = z_e[0]
                            nc.vector.tensor_copy(z_e[:, :, c:c + 1],
                                                  z_e[:, :, srcc:srcc + 1])
                        for c in range(-mO):          # z_o[m<0] = z_e[0]
                            nc.vector.tensor_copy(z_o[:, :, c:c + 1],
                                                  z_e[:, :, srcc:srcc + 1])
                    if i == NTILES - 1:
                        srco = T - 1 - mO  # col of z_o[m=T-1]
                        for c in range(T - mE, L):    # z_e[m>=T] = z_o[T-1]
                            nc.vector.tensor_copy(z_e[:, :, c:c + 1],
                                                  z_o[:, :, srco:srco + 1])
                        for c in range(T - mO, L):    # z_o[m>=T] = z_o[T-1]
                            nc.vector.tensor_copy(z_o[:, :, c:c + 1],
                                                  z_o[:, :, srco:srco + 1])

                    y_act = pool.tile([P, G, SA], f32r, name=f"ya{j}_{i}",
                                      tag="ya")
                    for g in range(G):
                        pd = psp.tile([P, SA], f32, name=f"pd{j}_{i}_{g}",
                                      tag="pd", bufs=2)
                        for jj in range(6):
                            nc.tensor.matmul(pd[:], dg[:, 12 + jj, :],
                                             z_e[:, g, jj:jj + SA],
                                             start=(jj == 0), stop=False)
                        for jj in range(6):
                            nc.tensor.matmul(pd[:], dg[:, 18 + jj, :],
                                             z_o[:, g, jj:jj + SA],
                                             start=False, stop=(jj == 5))
                        nc.scalar.activation(y_act[:, g, :], pd[:], AF.Copy)

                    # conv zero-padding: act output t<0 or t>=T must be 0
                    if i == 0 and d > 0:
                        nc.vector.memset(y_act[:, :, 0:d].bitcast(f32), 0.0)
                    if i == NTILES - 1 and d > 0:
                        nc.vector.memset(y_act[:, :, SA - d:SA].bitcast(f32), 0.0)

                    # ---- dilated conv 512x512 k=3
                    out_t = pool.tile([P, G, TC], f32r if j < 5 else f32,
                                      name=f"ot{j}_{i}", tag="ot")
                    for go in range(G):
                        pc = psp.tile([P, TC], f32, name=f"pc{j}_{i}_{go}",
                                      tag="pc", bufs=2)
                        first = True
                        for k in range(KER):
                            for gi in range(G):
                                nc.tensor.matmul(
                                    pc[:], wt[:, k, gi, go * P:(go + 1) * P],
                                    y_act[:, gi, k * d:k * d + TC],
                                    start=first, stop=(k == KER - 1 and gi == G - 1))
                                first = False
                        if res is not None:
                            if has_bias:
                                tmp = pool.tile([P, TC], f32, name=f"tb{j}_{i}_{go}",
                                                tag="tb", bufs=2)
                                nc.scalar.activation(tmp[:], pc[:], AF.Identity,
                                                     bias=bias_t[:, j, go:go + 1])
                                nc.vector.tensor_add(
                                    out_t[:, go, :], tmp[:],
                                    res_t[:, go, :].bitcast(f32))
                            else:
                                nc.vector.tensor_add(
                                    out_t[:, go, :], pc[:],
                                    res_t[:, go, :].bitcast(f32))
                        else:
                            if has_bias:
                                nc.scalar.activation(out_t[:, go, :], pc[:],
                                                     AF.Identity,
                                                     bias=bias_t[:, j, go:go + 1])
                            else:
                                nc.scalar.activation(out_t[:, go, :], pc[:],
                                                     AF.Copy)
                    nc.sync.dma_start(dst[:, :, t0:t0 + TC], out_t[:])
    nc.compile()
    return nc


def v3_host_mats():
    """TUP [128,128], TDN [128,3,128], IDN [128,128], all fp16."""
    fe, fo = _polyphase_filters()[:2]
    df = _kaiser_sinc_filter1d(0.25, 0.3, FILT_K)
    tup = np.zeros((P, P), dtype=np.float64)
    for r in range(P):
        if r % 2 == 0:
            for i in range(6):
                tup[r // 2 + i, r] = fe[i]
        else:
            for i in range(6):
                tup[(r - 1) // 2 + 1 + i, r] = fo[i]
    tdn = np.zeros((P, 3, P), dtype=np.float64)
    for k in range(3):
        for zr in range(P):
            for yr in range(P):
                jj = 128 * k + zr - 2 * yr - 1
                if 0 <= jj < FILT_K:
                    tdn[zr, k, yr] = df[jj]
    idn = np.eye(P)
    tdn20 = np.concatenate([tdn[:, 2, :], tdn[:, 0, :]], axis=1)
    return (tup.astype(np.float16), tdn.astype(np.float16),
            tdn20.astype(np.float16), idn.astype(np.float16))


def build_nc_v3(act_consts):
    """act_consts: [(a2pi_j, sb_j)] * 6, python floats."""
    from concourse import bacc, mybir, tile

    f16 = mybir.dt.float16
    f32 = mybir.dt.float32
    AF = mybir.ActivationFunctionType
    ALU = mybir.AluOpType

    nc = bacc.Bacc("TRN2", target_bir_lowering=False, debug=False,
                   num_devices=N_CORES)

    NB = T // P  # 64 row-blocks, block-major DRAM: [part, blk, C]
    x_ext = nc.dram_tensor("x", [P, NB, C], f16, kind="ExternalInput").ap()
    w_ext = nc.dram_tensor("w", [6, P, KER, G, C], f16,
                           kind="ExternalInput").ap()
    tup_ext = nc.dram_tensor("tup", [P, P], f16, kind="ExternalInput").ap()
    tdn_ext = nc.dram_tensor("tdn", [P, 3, P], f16,
                             kind="ExternalInput").ap()
    tdn20_ext = nc.dram_tensor("tdn20", [P, 2 * P], f16,
                               kind="ExternalInput").ap()
    idn_ext = nc.dram_tensor("idn", [P, P], f16, kind="ExternalInput").ap()
    out_ext = nc.dram_tensor("out", [P, NB, C], f32,
                             kind="ExternalOutput").ap()
    spill = [nc.dram_tensor(f"spill{j}", [P, NB, C], f16,
                            kind="Internal").ap()
             for j in range(5)]
    unit_src = [x_ext, spill[0], spill[1], spill[2], spill[3], spill[4]]
    unit_dst = [spill[0], spill[1], spill[2], spill[3], spill[4], out_ext]
    unit_res = [None, x_ext, None, spill[1], None, spill[3]]

    with tile.TileContext(nc) as tc:
        with tc.tile_pool(name="const", bufs=1) as cpool, \
             tc.tile_pool(name="wpool", bufs=2) as wpool, \
             tc.tile_pool(name="sbuf", bufs=3) as pool, \
             tc.tile_pool(name="psum", bufs=1, space="PSUM") as psp:

            tup = cpool.tile([P, P], f16)
            nc.sync.dma_start(tup[:], tup_ext[:])
            tdn = cpool.tile([P, 3, P], f16)
            nc.sync.dma_start(tdn[:], tdn_ext[:])
            tdn20 = cpool.tile([P, 2 * P], f16)
            nc.sync.dma_start(tdn20[:], tdn20_ext[:])
            idn = cpool.tile([P, P], f16)
            nc.sync.dma_start(idn[:], idn_ext[:])

            pend = {"v": None}  # pending conv (tile i-1, may cross units)
            def dma_sp(dst, srcap, chunks=4):
                """Bulk HBM load via gpsimd SWDGE: sprays descriptors across
                all 16 SDMA engines (the sync HWDGE ring serializes big
                HBM->SBUF loads onto one engine at ~25 GB/s)."""
                nc.gpsimd.dma_start(dst, srcap)

            def emit_conv(part):
                pv = pend["v"]
                if pv is None:
                    return
                jj, ii = pv["j"], pv["i"]
                ycm_p, wt_p, rt_p = pv["ycm"], pv["wt"], pv["res"]
                dst_p, dd = pv["dst"], pv["d"]
                t0p = ii * TCV
                if part == "early":
                    pv["ot"] = pool.tile([P, G, C], f16 if jj < 5 else f32,
                                         name=f"ot{jj}_{ii}",
                                         tag="ot" if jj < 5 else "ot32",
                                         bufs=2)
                    pv["pc"] = []
                    for tb in range(G):
                        pc = psp.tile([P, C], f32, name=f"pc{jj}_{ii}_{tb}",
                                      tag="pc", bufs=2)
                        pv["pc"].append(pc)
                        first = True
                        if rt_p is not None:
                            nc.tensor.matmul(pc[:], idn[:], rt_p[:, tb, :],
                                             start=True, stop=False)
                            first = False
                        for k in range(KER):
                            if tb == G - 1 and k == KER - 1:
                                break  # needs right halo; emitted in 'late'
                            for gi in range(G):
                                stop = (tb < G - 1 and k == KER - 1
                                        and gi == G - 1)
                                nc.tensor.matmul(
                                    pc[:],
                                    ycm_p[:, gi, 128 * tb + k * dd:
                                          128 * tb + k * dd + 128],
                                    wt_p[:, k, gi, :],
                                    start=first, stop=stop)
                                first = False
                        if tb < G - 1:
                            nc.scalar.activation(pv["ot"][:, tb, :], pc[:],
                                                 AF.Copy)
                else:  # 'late'
                    pc = pv["pc"][G - 1]
                    tb = G - 1
                    for gi in range(G):
                        nc.tensor.matmul(
                            pc[:],
                            ycm_p[:, gi, 128 * tb + 2 * dd:
                                  128 * tb + 2 * dd + 128],
                            wt_p[:, 2, gi, :],
                            start=False, stop=(gi == G - 1))
                    nc.scalar.activation(pv["ot"][:, tb, :], pc[:], AF.Copy)
                    bk = 4 * ii
                    nc.scalar.dma_start(dst_p[0:64, bk:bk + 4, :],
                                        pv["ot"][0:64])
                    nc.scalar.dma_start(dst_p[64:128, bk:bk + 4, :],
                                        pv["ot"][64:128])
                    pend["v"] = None

            for j in range(6):
                d = DILATIONS[j // 2] if j % 2 == 0 else 1
                a2pi, sb = act_consts[j]
                src, dst, res = unit_src[j], unit_dst[j], unit_res[j]

                wt = wpool.tile([P, KER, G, C], f16, name=f"wt{j}", tag="wt")
                nc.gpsimd.dma_start(wt[:], w_ext[j])

                prev_xe = None
                prev_z = None
                prev_y = None

                for i in range(NT):
                    t0 = i * TCV
                    # ---------------- loads ----------------
                    bk = 4 * i
                    xe = pool.tile([P, 5, C], f16, name=f"xe{j}_{i}",
                                   tag="xe", bufs=3)
                    if i == 0:
                        dma_sp(xe[6:128, 0:5, :], src[0:122, 0:5, :])
                        nc.gpsimd.dma_start(xe[0:6, 1:5, :],
                                            src[122:128, 0:4, :])
                        for p_ in range(6):
                            nc.sync.dma_start(xe[p_:p_ + 1, 0, :],
                                              src[0:1, 0, :])
                    elif i < NT - 1:
                        dma_sp(xe[6:128, 0:5, :], src[0:122, bk:bk + 5, :])
                        nc.gpsimd.dma_start(xe[0:6, 0:5, :],
                                            src[122:128, bk - 1:bk + 4, :])
                    else:
                        nc.vector.memset(xe[:, 4, :], 0.0)
                        dma_sp(xe[6:128, 0:4, :], src[0:122, bk:bk + 4, :])
                        nc.gpsimd.dma_start(xe[0:6, 0:5, :],
                                            src[122:128, bk - 1:bk + 4, :])
                        for p_ in range(6, 9):
                            nc.sync.dma_start(xe[p_:p_ + 1, 4, :],
                                              src[127:128, NB - 1, :])
                    xo = pool.tile([P, 4, C], f16, name=f"xo{j}_{i}",
                                   tag="xo", bufs=3)
                    if i < NT - 1:
                        dma_sp(xo[0:70, 0:4, :], src[58:128, bk:bk + 4, :],
                               chunks=3)
                        dma_sp(xo[70:128, 0:4, :],
                               src[0:58, bk + 1:bk + 5, :], chunks=2)
                    else:
                        nc.vector.memset(xo[:, 3, :], 0.0)
                        dma_sp(xo[0:70, 0:4, :], src[58:128, bk:bk + 4, :],
                               chunks=3)
                        dma_sp(xo[70:128, 0:3, :],
                               src[0:58, bk + 1:bk + 4, :], chunks=2)
                    rt = None
                    if res is not None:
                        rt = pool.tile([P, G, C], f16, name=f"rt{j}_{i}",
                                       tag="rt", bufs=3)
                        dma_sp(rt[:], res[:, bk:bk + 4, :], chunks=3)

                    ycm = pool.tile([P, G, TCV + 10], f16,
                                    name=f"ycm{j}_{i}", tag="ycm", bufs=3)
                    if i == 0:
                        nc.vector.memset(ycm[:, :, 0:d], 0.0)
                    else:
                        nc.sync.dma_start(ycm[:, :, 0:d],
                                          prev_y[:, :, TCV:TCV + d])

                    # ------------- up + snake (+ conv early) -------------
                    z_tm = pool.tile([P, 9, C], f16, name=f"z{j}_{i}",
                                     tag="z", bufs=2)
                    ms = list(range(0, 9)) if i == 0 else list(range(1, 9))
                    zq = []  # (m, pu, p_t) awaiting the z-add

                    def flush_z():
                        while zq:
                            m0, pu0, p0 = zq.pop(0)
                            nc.vector.tensor_add(z_tm[:, m0, :], pu0[:],
                                                 p0[:])

                    def snake(m):
                        # z = u + sin^2(a u)/b; fp32 magic-number range
                        # reduction (DVE ALU is fp32 internally, so the
                        # rounding must use the fp32 magic constant).
                        pu = psp.tile([P, C], f32, name=f"pu{j}_{i}_{m}",
                                      tag="pu", bufs=4)
                        mov = xe[:, m // 2, :] if m % 2 == 0 \
                            else xo[:, (m - 1) // 2, :]
                        nc.tensor.matmul(pu[:], tup[:], mov,
                                         start=True, stop=True)
                        q_t = pool.tile([P, C], f32, name=f"q{j}_{i}_{m}",
                                        tag="q", bufs=3)
                        nc.vector.tensor_scalar(q_t[:], pu[:], a2pi, None,
                                                op0=ALU.mult)
                        r_t = pool.tile([P, C], f32, name=f"r{j}_{i}_{m}",
                                        tag="r", bufs=3)
                        nc.vector.tensor_scalar(r_t[:], q_t[:], MAGIC,
                                                MAGIC, op0=ALU.add,
                                                op1=ALU.subtract)
                        f_t = pool.tile([P, C], f16, name=f"f{j}_{i}_{m}",
                                        tag="f", bufs=3)
                        nc.gpsimd.tensor_sub(f_t[:], q_t[:], r_t[:])
                        s_t = pool.tile([P, C], f32, name=f"s{j}_{i}_{m}",
                                        tag="s", bufs=3)
                        nc.scalar.activation(s_t[:], f_t[:], AF.Sin,
                                             bias=0.0,
                                             scale=float(2 * np.pi))
                        p_t = pool.tile([P, C], f32, name=f"p{j}_{i}_{m}",
                                        tag="p", bufs=3)
                        nc.scalar.activation(p_t[:], s_t[:], AF.Square,
                                             bias=0.0, scale=sb)
                        if zq:
                            m0, pu0, p0 = zq.pop(0)
                            nc.vector.tensor_add(z_tm[:, m0, :], pu0[:],
                                                 p0[:])
                        zq.append((m, pu, p_t))

                    nsplit = 5 if i == 0 else 4
                    for m in ms[:nsplit]:
                        snake(m)
                    emit_conv("early")
                    for m in ms[nsplit:]:
                        snake(m)
                    flush_z()

                    # edge clamps on z
                    if i == 0:
                        for p_ in range(6):
                            nc.sync.dma_start(z_tm[p_:p_ + 1, 0, :],
                                              z_tm[6:7, 0, :])
                    if i == NT - 1:
                        bz = pool.tile([P, C], f16, name=f"bz{j}",
                                       tag="bz", bufs=1)
                        nc.sync.dma_start(bz[0:1, :], z_tm[5:6, 8, :])
                        nc.gpsimd.partition_broadcast(bz[:], bz[0:1, :])
                        nc.sync.dma_start(z_tm[6:128, 8, :], bz[6:128, :])

                    # ---------------- down ----------------
                    # zblk = 2n+k couples (n,k); even zblk in {2,4,6} hits
                    # (k=2,n=(z-2)/2) and (k=0,n=z/2) -> one N=256 matmul
                    # with tdn20 = [tdn_k2 | tdn_k0] into adjacent psum cols.
                    for b in range(G):
                        pd = psp.tile([P, C], f32, name=f"pd{j}_{i}_{b}",
                                      tag="pd", bufs=2)
                        for zblk in range(9):
                            if zblk == 0 and i > 0:
                                zsrc = prev_z[:, 8, 128 * b:128 * (b + 1)]
                            else:
                                zsrc = z_tm[:, zblk, 128 * b:128 * (b + 1)]
                            start, stop = (zblk == 0), (zblk == 8)
                            if zblk in (2, 4, 6):
                                base = 128 * ((zblk - 2) // 2)
                                nc.tensor.matmul(pd[:, base:base + 256],
                                                 zsrc, tdn20[:],
                                                 start=start, stop=stop)
                            elif zblk == 0:
                                nc.tensor.matmul(pd[:, 0:128], zsrc,
                                                 tdn[:, 0, :],
                                                 start=start, stop=stop)
                            elif zblk == 8:
                                nc.tensor.matmul(pd[:, 384:512], zsrc,
                                                 tdn[:, 2, :],
                                                 start=start, stop=stop)
                            else:
                                n = (zblk - 1) // 2
                                nc.tensor.matmul(
                                    pd[:, 128 * n:128 * n + 128], zsrc,
                                    tdn[:, 1, :], start=start, stop=stop)
                        nc.scalar.activation(ycm[:, b, d:TCV + d], pd[:],
                                             AF.Copy)

                    if i == NT - 1:
                        nc.vector.memset(ycm[:, :, TCV + d:TCV + 2 * d], 0.0)
                    if pend["v"] is not None:
                        pv = pend["v"]
                        if pv["j"] == j:  # same unit: fill right halo
                            nc.sync.dma_start(
                                pv["ycm"][:, :,
                                          TCV + pv["d"]:TCV + 2 * pv["d"]],
                                ycm[:, :, d:2 * d])
                        emit_conv("late")

                    pend["v"] = {"j": j, "i": i, "ycm": ycm, "wt": wt,
                                 "res": rt, "dst": dst, "d": d}
                    prev_xe, prev_z, prev_y = xe, z_tm, ycm

            emit_conv("early")
            emit_conv("late")
    nc.compile()
    return nc

_NC_CACHE = {}
LAST_EXEC_NS = None


def kernel(**inputs):
    from concourse.bass_utils import run_bass_kernel_spmd

    x = np.asarray(inputs["x"], dtype=np.float32)          # [B, C, T]
    fe, fo, fde, fdo = _polyphase_filters()

    # diag filter matrices [P, 24, P] (v1 fallback)
    dg = np.zeros((P, 24, P), dtype=np.float32)
    coeffs = list(fe) + list(fo) + list(fde) + list(fdo)
    for idx, cf in enumerate(coeffs):
        np.fill_diagonal(dg[:, idx, :], cf)

    # weights [6, P, KER, G, C]: unit 2l -> w1_l, unit 2l+1 -> w2_l
    w_all = np.zeros((6, P, KER, G, C), dtype=np.float32)
    b_all = np.zeros((P, 6, G), dtype=np.float32)
    for l in range(3):
        for half, nm in ((0, "w1"), (1, "w2")):
            j = 2 * l + half
            w = np.asarray(inputs[f"{nm}_{l}"], dtype=np.float32)
            w_all[j] = w.transpose(1, 2, 0).reshape(G, P, KER, C).transpose(
                1, 2, 0, 3)
            b = np.asarray(inputs[f"b{half + 1}_{l}"], dtype=np.float32)
            b_all[:, j, :] = b.reshape(G, P).T

    sc = np.zeros((P, 6, G, 2), dtype=np.float32)
    for j in range(6):
        a = np.exp(np.asarray(inputs[f"alpha_{j}"], dtype=np.float64))
        bb = np.exp(np.asarray(inputs[f"beta_{j}"], dtype=np.float64))
        sc[:, j, :, 0] = (a / (2 * np.pi)).reshape(G, P).T
        sc[:, j, :, 1] = (1.0 / np.sqrt(bb + 1e-9)).reshape(G, P).T

    has_bias = bool(np.any(b_all != 0.0))
    uni = (not has_bias) and all(
        np.ptp(np.asarray(inputs[f"alpha_{j}"])) == 0
        and np.ptp(np.asarray(inputs[f"beta_{j}"])) == 0
        for j in range(6))
    global LAST_EXEC_NS
    if uni:
        act_consts = []
        for j in range(6):
            a = float(np.exp(np.asarray(inputs[f"alpha_{j}"]).ravel()[0]))
            bb = float(np.exp(np.asarray(inputs[f"beta_{j}"]).ravel()[0]))
            act_consts.append((a / (2 * np.pi), 1.0 / np.sqrt(bb + 1e-9)))
        key = ("v3", tuple(act_consts))
        if key not in _NC_CACHE:
            _NC_CACHE[key] = build_nc_v3(act_consts)
        nc = _NC_CACHE[key]
        tupm, tdnm, tdn20m, idnm = v3_host_mats()
        w16 = w_all.astype(np.float16)
        in_maps = []
        for bi in range(B):
            xb = x[bi].T.astype(np.float16)                       # [T, C]
            xb = np.ascontiguousarray(
                xb.reshape(T // P, P, C).transpose(1, 0, 2))      # [P,NB,C]
            in_maps.append({"x": xb, "w": w16, "tup": tupm, "tdn": tdnm,
                            "tdn20": tdn20m, "idn": idnm})
        res = run_bass_kernel_spmd(nc, in_maps, core_ids=list(range(N_CORES)))
        LAST_EXEC_NS = res.exec_time_ns
        out = np.empty((B, C, T), dtype=np.float32)
        for bi in range(B):
            ob = res.results[bi]["out"]                           # [P,NB,C]
            out[bi] = ob.transpose(1, 0, 2).reshape(T, C).T
        return out

    key = ("v1", has_bias)
    if key not in _NC_CACHE:
        _NC_CACHE[key] = _build_nc_v1(has_bias)
    nc = _NC_CACHE[key]
    in_maps = []
    for bi in range(B):
        xb = x[bi].reshape(G, P, T).transpose(1, 0, 2).copy()
        in_maps.append({"x": xb, "w": w_all, "dg": dg, "sc": sc,
                        "bias": b_all})
    res = run_bass_kernel_spmd(nc, in_maps, core_ids=list(range(N_CORES)))
    LAST_EXEC_NS = res.exec_time_ns
    out = np.empty((B, C, T), dtype=np.float32)
    for bi in range(B):
        out[bi] = res.results[bi]["out"].transpose(1, 0, 2).reshape(C, T)
    return out



# revision 19
# speedup vs baseline: 1.2883x; 1.2057x over previous
"""Trainium2 Bass kernel for BigVGAN AMPBlock1 (nn_AMPBlock1_81655918231624).

Strategy: data-parallel over batch (B=8 -> 1 sample per NeuronCore).
Per core, the whole block runs channel-major ([128 part = channel mod 128,
4 groups, T]) as 6 sequential "units" (act1d + dilated conv), with DRAM
spill between units:

  - up/down anti-alias FIRs: polyphase 6-tap filters as diagonal-matrix
    f32r matmuls on the TensorEngine (PSUM-accumulated).
  - snakebeta: x + sin^2(a*x)/b via range-reduced Sin on ScalarE:
      q = u * a/(2pi)            (DVE, PSUM read)
      r = (q + M) - M            (DVE, fp32 magic-number round)
      f = q - r  in [-0.5, 0.5]  (DVE)
      s = Sin(2pi * f)           (ScalarE; = +-sin(a*u), sign dies in square)
      p = Square(s * sqrt(1/b))  (ScalarE; = sin^2(a*u)/b)
      z = u + p                  (DVE)
  - 512x512 k=3 dilated convs: f32r matmuls, 4x4 channel blocks x 3 taps.
  - residual adds fused into the conv2 PSUM eviction.

Self-contained: shapes hardcoded; no sibling imports.
"""
import numpy as np

# ---------------------------------------------------------------- constants
B, C, T = 8, 512, 8192
G, P = 4, 128            # channel groups x partitions
KER = 3
DILATIONS = (1, 3, 5)
FILT_K = 12
TC = 256                 # v1 output columns per time-tile
TCV = 512                # v3 tile length
NT = T // TCV
NTILES = T // TC
MAGIC = 12582912.0       # 1.5 * 2**23: fp32 round-to-nearest-int
MAGIC16 = 1536.0         # 1.5 * 2**10: f16 round-to-nearest-int
N_CORES = 8


def _kaiser_sinc_filter1d(cutoff, half_width, kernel_size):
    even = kernel_size % 2 == 0
    half_size = kernel_size // 2
    delta_f = 4 * half_width
    A = 2.285 * (half_size - 1) * np.pi * delta_f + 7.95
    if A > 50.0:
        beta = 0.1102 * (A - 8.7)
    elif A >= 21.0:
        beta = 0.5842 * (A - 21) ** 0.4 + 0.07886 * (A - 21.0)
    else:
        beta = 0.0
    window = np.kaiser(kernel_size, beta)
    if even:
        time = np.arange(-half_size, half_size) + 0.5
    else:
        time = np.arange(kernel_size) - half_size
    if cutoff == 0:
        filt = np.zeros(kernel_size)
    else:
        filt = 2 * cutoff * window * np.sinc(2 * cutoff * time)
        filt = filt / np.sum(filt)
    return filt


def _polyphase_filters():
    """up even: u[2t]   = sum_i fe[i] * xc[t-3+i]
       up odd:  u[2t+1] = sum_i fo[i] * xc[t-2+i]
       down:    y[t] = sum_j fde[j]*ze[t-2+j] + fdo[j]*zo[t-3+j]"""
    up = _kaiser_sinc_filter1d(0.25, 0.3, FILT_K)
    dn = _kaiser_sinc_filter1d(0.25, 0.3, FILT_K)
    wf = 2.0 * up[::-1]
    return wf[0::2], wf[1::2], dn[1::2], dn[0::2]


def _build_nc_v1(has_bias):
    """Builds the Bacc graph. has_bias/has_act: enable general paths."""
    from concourse import bacc, mybir, tile

    f32r = mybir.dt.float32r
    f32 = mybir.dt.float32
    AF = mybir.ActivationFunctionType
    ALU = mybir.AluOpType

    nc = bacc.Bacc("TRN2", target_bir_lowering=False, debug=False,
                   num_devices=N_CORES)

    x_ext = nc.dram_tensor("x", [P, G, T], f32r, kind="ExternalInput").ap()
    # weights: [unit, part=ci%128, k, gi, co(512)]
    w_ext = nc.dram_tensor("w", [6, P, KER, G, C], f32r,
                           kind="ExternalInput").ap()
    # 24 diagonal filter matrices, partition-major: [part, idx, 128]
    dg_ext = nc.dram_tensor("dg", [P, 24, P], f32r, kind="ExternalInput").ap()
    # per-act scalars: [part, unit, g, {a/(2pi), sqrt(1/(b+eps))}]
    sc_ext = nc.dram_tensor("sc", [P, 6, G, 2], f32, kind="ExternalInput").ap()
    bias_ext = nc.dram_tensor("bias", [P, 6, G], f32, kind="ExternalInput").ap()
    out_ext = nc.dram_tensor("out", [P, G, T], f32, kind="ExternalOutput").ap()

    spill = [nc.dram_tensor(f"spill{j}", [P, G, T], f32r, kind="Internal").ap()
             for j in range(5)]
    # unit j: input src, output dst, residual (None if no add)
    unit_src = [x_ext, spill[0], spill[1], spill[2], spill[3], spill[4]]
    unit_dst = [spill[0], spill[1], spill[2], spill[3], spill[4], out_ext]
    unit_res = [None, x_ext, None, spill[1], None, spill[3]]

    with tile.TileContext(nc) as tc:
        with tc.tile_pool(name="const", bufs=1) as cpool, \
             tc.tile_pool(name="wpool", bufs=2) as wpool, \
             tc.tile_pool(name="sbuf", bufs=3) as pool, \
             tc.tile_pool(name="psum", bufs=1, space="PSUM") as psp:

            dg = cpool.tile([P, 24, P], f32r)
            nc.sync.dma_start(dg[:], dg_ext[:])
            sc = cpool.tile([P, 6, G, 2], f32)
            nc.sync.dma_start(sc[:], sc_ext[:])
            bias_t = cpool.tile([P, 6, G], f32)
            if has_bias:
                nc.sync.dma_start(bias_t[:], bias_ext[:])

            for j in range(6):
                d = DILATIONS[j // 2] if j % 2 == 0 else 1
                h = d + 6                 # left halo in x (col0 = t0 - h)
                XL = TC + 2 * d + 12      # x tile length
                L = TC + 2 * d + 6        # phase (u/z) length, even
                SA = TC + 2 * d           # act output length
                src, dst, res = unit_src[j], unit_dst[j], unit_res[j]

                wt = wpool.tile([P, KER, G, C], f32r, name=f"wt{j}", tag="wt")
                nc.sync.dma_start(wt[:], w_ext[j])

                for i in range(NTILES):
                    t0 = i * TC
                    lo = t0 - h              # absolute x index of x_in col 0
                    x_in = pool.tile([P, G, XL], f32r, name=f"xin{j}_{i}",
                                     tag="xin", bufs=2)
                    # ---- input DMA with edge clamping
                    lo_c = max(lo, 0)
                    hi_c = min(lo + XL, T)
                    nc.sync.dma_start(x_in[:, :, lo_c - lo:hi_c - lo],
                                      src[:, :, lo_c:hi_c])
                    for c in range(lo_c - lo):                    # left clamp
                        nc.sync.dma_start(x_in[:, :, c:c + 1], src[:, :, 0:1])
                    for c in range(hi_c - lo, XL):               # right clamp
                        nc.sync.dma_start(x_in[:, :, c:c + 1],
                                          src[:, :, T - 1:T])

                    if res is not None:
                        res_t = pool.tile([P, G, TC], f32r,
                                          name=f"res{j}_{i}", tag="res")
                        nc.sync.dma_start(res_t[:], res[:, :, t0:t0 + TC])

                    # ---- act1d: up (diag matmuls) + snake + down
                    z_ph = []
                    for ph, base in ((0, 0), (1, 6)):
                        z_t = pool.tile([P, G, L], f32r,
                                        name=f"z{j}_{i}_{ph}", tag=f"z{ph}")
                        for g in range(G):
                            pu = psp.tile([P, L], f32, name=f"pu{j}_{i}_{ph}_{g}",
                                          tag="pu", bufs=4)
                            for ii in range(6):
                                nc.tensor.matmul(pu[:], dg[:, base + ii, :],
                                                 x_in[:, g, ii + 1:ii + 1 + L],
                                                 start=(ii == 0), stop=(ii == 5))
                            q_t = pool.tile([P, L], f32, name=f"q{j}_{i}_{ph}_{g}",
                                            tag="q", bufs=2)
                            nc.vector.tensor_scalar_mul(q_t[:], pu[:],
                                                        sc[:, j, g, 0:1])
                            r_t = pool.tile([P, L], f32, name=f"r{j}_{i}_{ph}_{g}",
                                            tag="r", bufs=2)
                            nc.vector.tensor_scalar(r_t[:], q_t[:], MAGIC, MAGIC,
                                                    op0=ALU.add, op1=ALU.subtract)
                            f_t = pool.tile([P, L], f32, name=f"f{j}_{i}_{ph}_{g}",
                                            tag="f", bufs=2)
                            nc.vector.tensor_sub(f_t[:], q_t[:], r_t[:])
                            s_t = pool.tile([P, L], f32, name=f"s{j}_{i}_{ph}_{g}",
                                            tag="s", bufs=2)
                            nc.scalar.activation(s_t[:], f_t[:], AF.Sin,
                                                 bias=0.0, scale=float(2 * np.pi))
                            p_t = pool.tile([P, L], f32, name=f"p{j}_{i}_{ph}_{g}",
                                            tag="p", bufs=2)
                            nc.scalar.activation(p_t[:], s_t[:], AF.Square,
                                                 bias=0.0, scale=sc[:, j, g, 1:2])
                            nc.vector.tensor_add(z_t[:, g, :], pu[:], p_t[:])
                        z_ph.append(z_t)
                    z_e, z_o = z_ph

                    # ---- z edge clamping (replicate-pad semantics of down)
                    # z_e col c is z-phase-e index mE + c, mE = t0 - d - 2
                    # z_o col c is z-phase-o index mO + c, mO = t0 - d - 3
                    mE = t0 - d - 2
                    mO = t0 - d - 3
                    if i == 0:
                        srcc = -mE        # col of z_e[m=0]
                        for c in range(-mE):          # z_e[m<0] = z_e[0]
                            nc.vector.tensor_copy(z_e[:, :, c:c + 1],
                                                  z_e[:, :, srcc:srcc + 1])
                        for c in range(-mO):          # z_o[m<0] = z_e[0]
                            nc.vector.tensor_copy(z_o[:, :, c:c + 1],
                                                  z_e[:, :, srcc:srcc + 1])
                    if i == NTILES - 1:
                        srco = T - 1 - mO  # col of z_o[m=T-1]
                        for c in range(T - mE, L):    # z_e[m>=T] = z_o[T-1]
                            nc.vector.tensor_copy(z_e[:, :, c:c + 1],
                                                  z_o[:, :, srco:srco + 1])
                        for c in range(T - mO, L):    # z_o[m>=T] = z_o[T-1]
                            nc.vector.tensor_copy(z_o[:, :, c:c + 1],
                                                  z_o[:, :, srco:srco + 1])

                    y_act = pool.tile([P, G, SA], f32r, name=f"ya{j}_{i}",
                                      tag="ya")
                    for g in range(G):
                        pd = psp.tile([P, SA], f32, name=f"pd{j}_{i}_{g}",
                                      tag="pd", bufs=2)
                        for jj in range(6):
                            nc.tensor.matmul(pd[:], dg[:, 12 + jj, :],
                                             z_e[:, g, jj:jj + SA],
                                             start=(jj == 0), stop=False)
                        for jj in range(6):
                            nc.tensor.matmul(pd[:], dg[:, 18 + jj, :],
                                             z_o[:, g, jj:jj + SA],
                                             start=False, stop=(jj == 5))
                        nc.scalar.activation(y_act[:, g, :], pd[:], AF.Copy)

                    # conv zero-padding: act output t<0 or t>=T must be 0
                    if i == 0 and d > 0:
                        nc.vector.memset(y_act[:, :, 0:d].bitcast(f32), 0.0)
                    if i == NTILES - 1 and d > 0:
                        nc.vector.memset(y_act[:, :, SA - d:SA].bitcast(f32), 0.0)

                    # ---- dilated conv 512x512 k=3
                    out_t = pool.tile([P, G, TC], f32r if j < 5 else f32,
                                      name=f"ot{j}_{i}", tag="ot")
                    for go in range(G):
                        pc = psp.tile([P, TC], f32, name=f"pc{j}_{i}_{go}",
                                      tag="pc", bufs=2)
                        first = True
                        for k in range(KER):
                            for gi in range(G):
                                nc.tensor.matmul(
                                    pc[:], wt[:, k, gi, go * P:(go + 1) * P],
                                    y_act[:, gi, k * d:k * d + TC],
                                    start=first, stop=(k == KER - 1 and gi == G - 1))
                                first = False
                        if res is not None:
                            if has_bias:
                                tmp = pool.tile([P, TC], f32, name=f"tb{j}_{i}_{go}",
                                                tag="tb", bufs=2)
                                nc.scalar.activation(tmp[:], pc[:], AF.Identity,
                                                     bias=bias_t[:, j, go:go + 1])
                                nc.vector.tensor_add(
                                    out_t[:, go, :], tmp[:],
                                    res_t[:, go, :].bitcast(f32))
                            else:
                                nc.vector.tensor_add(
                                    out_t[:, go, :], pc[:],
                                    res_t[:, go, :].bitcast(f32))
                        else:
                            if has_bias:
                                nc.scalar.activation(out_t[:, go, :], pc[:],
                                                     AF.Identity,
                                                     bias=bias_t[:, j, go:go + 1])
                            else:
                                nc.scalar.activation(out_t[:, go, :], pc[:],
                                                     AF.Copy)
                    nc.sync.dma_start(dst[:, :, t0:t0 + TC], out_t[:])
    nc.compile()
    return nc


def v3_host_mats():
    """TUP [128,128], TDN [128,3,128], IDN [128,128], all fp16."""
    fe, fo = _polyphase_filters()[:2]
    df = _kaiser_sinc_filter1d(0.25, 0.3, FILT_K)
    tup = np.zeros((P, P), dtype=np.float64)
    for r in range(P):
        if r % 2 == 0:
            for i in range(6):
                tup[r // 2 + i, r] = fe[i]
        else:
            for i in range(6):
                tup[(r - 1) // 2 + 1 + i, r] = fo[i]
    tdn = np.zeros((P, 3, P), dtype=np.float64)
    for k in range(3):
        for zr in range(P):
            for yr in range(P):
                jj = 128 * k + zr - 2 * yr - 1
                if 0 <= jj < FILT_K:
                    tdn[zr, k, yr] = df[jj]
    idn = np.eye(P)
    tdn20 = np.concatenate([tdn[:, 2, :], tdn[:, 0, :]], axis=1)
    return (tup.astype(np.float16), tdn.astype(np.float16),
            tdn20.astype(np.float16), idn.astype(np.float16))


def build_nc_v3(act_consts):
    """act_consts: [(a2pi_j, sb_j)] * 6, python floats."""
    from concourse import bacc, mybir, tile

    f16 = mybir.dt.float16
    f32 = mybir.dt.float32
    AF = mybir.ActivationFunctionType
    ALU = mybir.AluOpType

    nc = bacc.Bacc("TRN2", target_bir_lowering=False, debug=False,
                   num_devices=N_CORES)

    NB = T // P  # 64 row-blocks, block-major DRAM: [part, blk, C]
    x_ext = nc.dram_tensor("x", [P, NB, C], f16, kind="ExternalInput").ap()
    w_ext = nc.dram_tensor("w", [6, P, KER, G, C], f16,
                           kind="ExternalInput").ap()
    tup_ext = nc.dram_tensor("tup", [P, P], f16, kind="ExternalInput").ap()
    tdn_ext = nc.dram_tensor("tdn", [P, 3, P], f16,
                             kind="ExternalInput").ap()
    tdn20_ext = nc.dram_tensor("tdn20", [P, 2 * P], f16,
                               kind="ExternalInput").ap()
    idn_ext = nc.dram_tensor("idn", [P, P], f16, kind="ExternalInput").ap()
    out_ext = nc.dram_tensor("out", [P, NB, C], f32,
                             kind="ExternalOutput").ap()
    spill = [nc.dram_tensor(f"spill{j}", [P, NB, C], f16,
                            kind="Internal").ap()
             for j in range(5)]
    unit_src = [x_ext, spill[0], spill[1], spill[2], spill[3], spill[4]]
    unit_dst = [spill[0], spill[1], spill[2], spill[3], spill[4], out_ext]
    unit_res = [None, x_ext, None, spill[1], None, spill[3]]

    with tile.TileContext(nc) as tc:
        with tc.tile_pool(name="const", bufs=1) as cpool, \
             tc.tile_pool(name="wpool", bufs=2) as wpool, \
             tc.tile_pool(name="sbuf", bufs=3) as pool, \
             tc.tile_pool(name="psum", bufs=1, space="PSUM") as psp:

            tup = cpool.tile([P, P], f16)
            nc.sync.dma_start(tup[:], tup_ext[:])
            tdn = cpool.tile([P, 3, P], f16)
            nc.sync.dma_start(tdn[:], tdn_ext[:])
            tdn20 = cpool.tile([P, 2 * P], f16)
            nc.sync.dma_start(tdn20[:], tdn20_ext[:])
            idn = cpool.tile([P, P], f16)
            nc.sync.dma_start(idn[:], idn_ext[:])

            pend = {"v": None}  # pending conv (tile i-1, may cross units)
            def dma_sp(dst, srcap, chunks=4):
                """Bulk HBM load via gpsimd SWDGE: sprays descriptors across
                all 16 SDMA engines (the sync HWDGE ring serializes big
                HBM->SBUF loads onto one engine at ~25 GB/s)."""
                nc.gpsimd.dma_start(dst, srcap)

            def emit_conv(part):
                pv = pend["v"]
                if pv is None:
                    return
                jj, ii = pv["j"], pv["i"]
                ycm_p, wt_p, rt_p = pv["ycm"], pv["wt"], pv["res"]
                dst_p, dd = pv["dst"], pv["d"]
                t0p = ii * TCV
                if part == "early":
                    pv["ot"] = pool.tile([P, G, C], f16 if jj < 5 else f32,
                                         name=f"ot{jj}_{ii}",
                                         tag="ot" if jj < 5 else "ot32",
                                         bufs=2)
                    pv["pc"] = []
                    for tb in range(G):
                        pc = psp.tile([P, C], f32, name=f"pc{jj}_{ii}_{tb}",
                                      tag="pc", bufs=2)
                        pv["pc"].append(pc)
                        first = True
                        if rt_p is not None:
                            nc.tensor.matmul(pc[:], idn[:], rt_p[:, tb, :],
                                             start=True, stop=False)
                            first = False
                        for k in range(KER):
                            if tb == G - 1 and k == KER - 1:
                                break  # needs right halo; emitted in 'late'
                            for gi in range(G):
                                stop = (tb < G - 1 and k == KER - 1
                                        and gi == G - 1)
                                nc.tensor.matmul(
                                    pc[:],
                                    ycm_p[:, gi, 128 * tb + k * dd:
                                          128 * tb + k * dd + 128],
                                    wt_p[:, k, gi, :],
                                    start=first, stop=stop)
                                first = False
                        if tb < G - 1:
                            nc.scalar.activation(pv["ot"][:, tb, :], pc[:],
                                                 AF.Copy)
                else:  # 'late'
                    pc = pv["pc"][G - 1]
                    tb = G - 1
                    for gi in range(G):
                        nc.tensor.matmul(
                            pc[:],
                            ycm_p[:, gi, 128 * tb + 2 * dd:
                                  128 * tb + 2 * dd + 128],
                            wt_p[:, 2, gi, :],
                            start=False, stop=(gi == G - 1))
                    nc.scalar.activation(pv["ot"][:, tb, :], pc[:], AF.Copy)
                    bk = 4 * ii
                    nc.scalar.dma_start(dst_p[0:64, bk:bk + 4, :],
                                        pv["ot"][0:64])
                    nc.scalar.dma_start(dst_p[64:128, bk:bk + 4, :],
                                        pv["ot"][64:128])
                    pend["v"] = None

            for j in range(6):
                d = DILATIONS[j // 2] if j % 2 == 0 else 1
                a2pi, sb = act_consts[j]
                src, dst, res = unit_src[j], unit_dst[j], unit_res[j]

                wt = wpool.tile([P, KER, G, C], f16, name=f"wt{j}", tag="wt")
                nc.gpsimd.dma_start(wt[:], w_ext[j])

                # Software pipeline: iteration i runs snake for tile s=i
                # (up matmuls + pointwise into z) and down+conv for tile
                # D=i-1 (whose z finished last iteration).  The PE never
                # waits on the pointwise chain, and the pointwise engines
                # always have the next tile's ups issued early.
                z_s1 = None     # z of tile i-1 (down target this iter)
                z_s2 = None     # z of tile i-2 (block-8 halo for down)
                prev_y = None   # ycm of tile i-2 (left halo source)
                rt_s1 = None    # residual tile for tile i-1

                def load_tile(ii):
                    """Issue the HBM loads for tile ii; returns (xe,xo,rt)."""
                    bk = 4 * ii
                    xe = pool.tile([P, 5, C], f16, name=f"xe{j}_{ii}",
                                   tag="xe", bufs=3)
                    if ii == 0:
                        dma_sp(xe[6:128, 0:5, :], src[0:122, 0:5, :])
                        nc.gpsimd.dma_start(xe[0:6, 1:5, :],
                                            src[122:128, 0:4, :])
                        for p_ in range(6):
                            nc.sync.dma_start(xe[p_:p_ + 1, 0, :],
                                              src[0:1, 0, :])
                    elif ii < NT - 1:
                        dma_sp(xe[6:128, 0:5, :],
                               src[0:122, bk:bk + 5, :])
                        nc.gpsimd.dma_start(
                            xe[0:6, 0:5, :],
                            src[122:128, bk - 1:bk + 4, :])
                    else:
                        nc.vector.memset(xe[:, 4, :], 0.0)
                        dma_sp(xe[6:128, 0:4, :],
                               src[0:122, bk:bk + 4, :])
                        nc.gpsimd.dma_start(
                            xe[0:6, 0:5, :],
                            src[122:128, bk - 1:bk + 4, :])
                        for p_ in range(6, 9):
                            nc.sync.dma_start(xe[p_:p_ + 1, 4, :],
                                              src[127:128, NB - 1, :])
                    xo = pool.tile([P, 4, C], f16, name=f"xo{j}_{ii}",
                                   tag="xo", bufs=3)
                    if ii < NT - 1:
                        dma_sp(xo[0:70, 0:4, :],
                               src[58:128, bk:bk + 4, :])
                        dma_sp(xo[70:128, 0:4, :],
                               src[0:58, bk + 1:bk + 5, :])
                    else:
                        nc.vector.memset(xo[:, 3, :], 0.0)
                        dma_sp(xo[0:70, 0:4, :],
                               src[58:128, bk:bk + 4, :])
                        dma_sp(xo[70:128, 0:3, :],
                               src[0:58, bk + 1:bk + 4, :])
                    rt = None
                    if res is not None:
                        rt = pool.tile([P, G, C], f16, name=f"rt{j}_{ii}",
                                       tag="rt", bufs=3)
                        dma_sp(rt[:], res[:, bk:bk + 4, :])
                    return xe, xo, rt

                cur = load_tile(0)

                for i in range(NT + 1):
                    # ---------------- snake for tile s=i ------------------
                    z_tm = None
                    rt = None
                    if i < NT:
                        xe, xo, rt = cur
                        if i + 1 < NT:
                            nxt = load_tile(i + 1)

                        z_tm = pool.tile([P, 9, C], f16, name=f"z{j}_{i}",
                                         tag="z", bufs=3)
                        ms = list(range(0, 9)) if i == 0 else list(range(1, 9))
                        zq = []  # (m, pu, p_t) awaiting the z-add

                        def snake(m, z_dst, xe_t, xo_t, ii):
                            # z = u + sin^2(a u)/b; fp32 magic-number range
                            # reduction (engine ALUs are fp32 internally).
                            pu = psp.tile([P, C], f32,
                                          name=f"pu{j}_{ii}_{m}",
                                          tag="pu", bufs=4)
                            mov = xe_t[:, m // 2, :] if m % 2 == 0 \
                                else xo_t[:, (m - 1) // 2, :]
                            nc.tensor.matmul(pu[:], tup[:], mov,
                                             start=True, stop=True)
                            q_t = pool.tile([P, C], f32,
                                            name=f"q{j}_{ii}_{m}",
                                            tag="q", bufs=3)
                            nc.vector.tensor_scalar(q_t[:], pu[:], a2pi,
                                                    None, op0=ALU.mult)
                            r_t = pool.tile([P, C], f32,
                                            name=f"r{j}_{ii}_{m}",
                                            tag="r", bufs=3)
                            nc.vector.tensor_scalar(r_t[:], q_t[:], MAGIC,
                                                    MAGIC, op0=ALU.add,
                                                    op1=ALU.subtract)
                            f_t = pool.tile([P, C], f16,
                                            name=f"f{j}_{ii}_{m}",
                                            tag="f", bufs=3)
                            nc.gpsimd.tensor_sub(f_t[:], q_t[:], r_t[:])
                            s_t = pool.tile([P, C], f32,
                                            name=f"s{j}_{ii}_{m}",
                                            tag="s", bufs=3)
                            nc.scalar.activation(s_t[:], f_t[:], AF.Sin,
                                                 bias=0.0,
                                                 scale=float(2 * np.pi))
                            p_t = pool.tile([P, C], f32,
                                            name=f"p{j}_{ii}_{m}",
                                            tag="p", bufs=3)
                            nc.scalar.activation(p_t[:], s_t[:], AF.Square,
                                                 bias=0.0, scale=sb)
                            if zq:
                                m0, pu0, p0 = zq.pop(0)
                                nc.vector.tensor_add(z_dst[:, m0, :],
                                                     pu0[:], p0[:])
                            zq.append((m, pu, p_t))

                        for m in ms:
                            snake(m, z_tm, xe, xo, i)
                        while zq:
                            m0, pu0, p0 = zq.pop(0)
                            nc.vector.tensor_add(z_tm[:, m0, :], pu0[:],
                                                 p0[:])

                        if i == 0:
                            for p_ in range(6):
                                nc.sync.dma_start(z_tm[p_:p_ + 1, 0, :],
                                                  z_tm[6:7, 0, :])
                        if i == NT - 1:
                            bz = pool.tile([P, C], f16, name=f"bz{j}",
                                           tag="bz", bufs=1)
                            nc.sync.dma_start(bz[0:1, :], z_tm[5:6, 8, :])
                            nc.gpsimd.partition_broadcast(bz[:], bz[0:1, :])
                            nc.sync.dma_start(z_tm[6:128, 8, :],
                                              bz[6:128, :])

                    # ------------- down + conv for tile D=i-1 -------------
                    if i >= 1:
                        D = i - 1
                        ycm = pool.tile([P, G, TCV + 10], f16,
                                        name=f"ycm{j}_{D}", tag="ycm",
                                        bufs=3)
                        if D == 0:
                            nc.vector.memset(ycm[:, :, 0:d], 0.0)
                        else:
                            nc.sync.dma_start(ycm[:, :, 0:d],
                                              prev_y[:, :, TCV:TCV + d])

                        # zblk = 2n+k couples (n,k); even zblk in {2,4,6}
                        # -> one N=256 matmul with tdn20 = [tdn_k2|tdn_k0].
                        for b in range(G):
                            pd = psp.tile([P, C], f32,
                                          name=f"pd{j}_{D}_{b}",
                                          tag="pd", bufs=2)
                            for zblk in range(9):
                                if zblk == 0 and D > 0:
                                    zsrc = z_s2[:, 8, 128 * b:128 * (b + 1)]
                                else:
                                    zsrc = z_s1[:, zblk,
                                                128 * b:128 * (b + 1)]
                                start, stop = (zblk == 0), (zblk == 8)
                                if zblk in (2, 4, 6):
                                    base = 128 * ((zblk - 2) // 2)
                                    nc.tensor.matmul(
                                        pd[:, base:base + 256], zsrc,
                                        tdn20[:], start=start, stop=stop)
                                elif zblk == 0:
                                    nc.tensor.matmul(pd[:, 0:128], zsrc,
                                                     tdn[:, 0, :],
                                                     start=start, stop=stop)
                                elif zblk == 8:
                                    nc.tensor.matmul(pd[:, 384:512], zsrc,
                                                     tdn[:, 2, :],
                                                     start=start, stop=stop)
                                else:
                                    n = (zblk - 1) // 2
                                    nc.tensor.matmul(
                                        pd[:, 128 * n:128 * n + 128], zsrc,
                                        tdn[:, 1, :], start=start,
                                        stop=stop)
                            nc.scalar.activation(ycm[:, b, d:TCV + d],
                                                 pd[:], AF.Copy)

                        if D == NT - 1:
                            nc.vector.memset(
                                ycm[:, :, TCV + d:TCV + 2 * d], 0.0)
                        if pend["v"] is not None:
                            pv = pend["v"]
                            if pv["j"] == j:  # same unit: fill right halo
                                nc.sync.dma_start(
                                    pv["ycm"][:, :,
                                              TCV + pv["d"]:
                                              TCV + 2 * pv["d"]],
                                    ycm[:, :, d:2 * d])
                            emit_conv("late")

                        pend["v"] = {"j": j, "i": D, "ycm": ycm, "wt": wt,
                                     "res": rt_s1, "dst": dst, "d": d}
                        emit_conv("early")
                        prev_y = ycm

                    z_s2, z_s1 = z_s1, z_tm
                    rt_s1 = rt
                    if i + 1 < NT:
                        cur = nxt

            emit_conv("late")
    nc.compile()
    return nc

_NC_CACHE = {}
LAST_EXEC_NS = None


def kernel(**inputs):
    from concourse.bass_utils import run_bass_kernel_spmd

    x = np.asarray(inputs["x"], dtype=np.float32)          # [B, C, T]
    fe, fo, fde, fdo = _polyphase_filters()

    # diag filter matrices [P, 24, P] (v1 fallback)
    dg = np.zeros((P, 24, P), dtype=np.float32)
    coeffs = list(fe) + list(fo) + list(fde) + list(fdo)
    for idx, cf in enumerate(coeffs):
        np.fill_diagonal(dg[:, idx, :], cf)

    # weights [6, P, KER, G, C]: unit 2l -> w1_l, unit 2l+1 -> w2_l
    w_all = np.zeros((6, P, KER, G, C), dtype=np.float32)
    b_all = np.zeros((P, 6, G), dtype=np.float32)
    for l in range(3):
        for half, nm in ((0, "w1"), (1, "w2")):
            j = 2 * l + half
            w = np.asarray(inputs[f"{nm}_{l}"], dtype=np.float32)
            w_all[j] = w.transpose(1, 2, 0).reshape(G, P, KER, C).transpose(
                1, 2, 0, 3)
            b = np.asarray(inputs[f"b{half + 1}_{l}"], dtype=np.float32)
            b_all[:, j, :] = b.reshape(G, P).T

    sc = np.zeros((P, 6, G, 2), dtype=np.float32)
    for j in range(6):
        a = np.exp(np.asarray(inputs[f"alpha_{j}"], dtype=np.float64))
        bb = np.exp(np.asarray(inputs[f"beta_{j}"], dtype=np.float64))
        sc[:, j, :, 0] = (a / (2 * np.pi)).reshape(G, P).T
        sc[:, j, :, 1] = (1.0 / np.sqrt(bb + 1e-9)).reshape(G, P).T

    has_bias = bool(np.any(b_all != 0.0))
    uni = (not has_bias) and all(
        np.ptp(np.asarray(inputs[f"alpha_{j}"])) == 0
        and np.ptp(np.asarray(inputs[f"beta_{j}"])) == 0
        for j in range(6))
    global LAST_EXEC_NS
    if uni:
        act_consts = []
        for j in range(6):
            a = float(np.exp(np.asarray(inputs[f"alpha_{j}"]).ravel()[0]))
            bb = float(np.exp(np.asarray(inputs[f"beta_{j}"]).ravel()[0]))
            act_consts.append((a / (2 * np.pi), 1.0 / np.sqrt(bb + 1e-9)))
        key = ("v3", tuple(act_consts))
        if key not in _NC_CACHE:
            _NC_CACHE[key] = build_nc_v3(act_consts)
        nc = _NC_CACHE[key]
        tupm, tdnm, tdn20m, idnm = v3_host_mats()
        w16 = w_all.astype(np.float16)
        in_maps = []
        for bi in range(B):
            xb = x[bi].T.astype(np.float16)                       # [T, C]
            xb = np.ascontiguousarray(
                xb.reshape(T // P, P, C).transpose(1, 0, 2))      # [P,NB,C]
            in_maps.append({"x": xb, "w": w16, "tup": tupm, "tdn": tdnm,
                            "tdn20": tdn20m, "idn": idnm})
        res = run_bass_kernel_spmd(nc, in_maps, core_ids=list(range(N_CORES)))
        LAST_EXEC_NS = res.exec_time_ns
        out = np.empty((B, C, T), dtype=np.float32)
        for bi in range(B):
            ob = res.results[bi]["out"]                           # [P,NB,C]
            out[bi] = ob.transpose(1, 0, 2).reshape(T, C).T
        return out

    key = ("v1", has_bias)
    if key not in _NC_CACHE:
        _NC_CACHE[key] = _build_nc_v1(has_bias)
    nc = _NC_CACHE[key]
    in_maps = []
    for bi in range(B):
        xb = x[bi].reshape(G, P, T).transpose(1, 0, 2).copy()
        in_maps.append({"x": xb, "w": w_all, "dg": dg, "sc": sc,
                        "bias": b_all})
    res = run_bass_kernel_spmd(nc, in_maps, core_ids=list(range(N_CORES)))
    LAST_EXEC_NS = res.exec_time_ns
    out = np.empty((B, C, T), dtype=np.float32)
    for bi in range(B):
        out[bi] = res.results[bi]["out"].transpose(1, 0, 2).reshape(C, T)
    return out

